# revision 13
# baseline (speedup 1.0000x reference)
"""Trainium2 Bass kernel for nn_MultiHeadAttention_52192442581071.

Reference math:
    qp  = query @ Wq.T                                   [bs, D]
    kp  = keys @ Wk.T ; vp = values @ Wv.T               [sl, bs, D]
    dot = (qp * kp).reshape(sl,bs,H,64).sum(-1)/8        [sl, bs, H]
    w   = log_softmax(dot, axis=0)
    attn= (w[...,None] * vp.reshape(sl,bs,H,64)).sum(0)  [bs, H, 64]
    out = attn.reshape(bs,-1) @ Wo.T                     [bs, D]

Algebraic restructuring used here (exact in exact arithmetic):
    r[b,h,:]   = sum_{j in head h} qp[b,j] * Wk[j,:] / 8          (small)
    dot[s,b,h] = keys[s,b,:] . r[b,h,:]                            (matmul)
    Since w = dot - lse (log_softmax is linear in dot given lse):
    z[b,h,:]   = sum_s w * values[s,b,:] = P[b,h,:] - lse[b,h]*V[b,:]
        P[b,h,:] = sum_s dot[s,b,h] * values[s,b,:]                (matmul)
        V[b,:]   = sum_s values[s,b,:]          (ones-column trick)
    attn[b,h,n] = z[b,h,:] . Wv[h*64+n,:]                          (small)
    out = attn @ Wo.T                                              (small)

This removes the two big [sl*bs, D] x [D, D] projections (275 GFLOP ->
~7 GFLOP) and makes the kernel HBM-bound: each core streams its shard of
keys/values (64 MiB) once.

Sharding: data-parallel over bs. Core i handles batch rows [4i, 4i+4).
No collectives; same program on all 8 cores with different inputs.
"""

import sys

if "/opt/trn_rl_repo" not in sys.path:
    sys.path.insert(0, "/opt/trn_rl_repo")

import numpy as np

import concourse.bass as bass
import concourse.mybir as mybir
import concourse.tile as tile
from concourse import bacc, bass_utils

# Problem constants (hardcoded per contract)
H = 16          # num heads
NHID = 64
D = 1024
SL = 2048
BS = 32
NCORES = 8
B = BS // NCORES  # 4 local batch elements per core

FP32 = mybir.dt.float32
FP32R = mybir.dt.float32r
Copy = mybir.ActivationFunctionType.Copy
Exp = mybir.ActivationFunctionType.Exp
Ln = mybir.ActivationFunctionType.Ln
X = mybir.AxisListType.X

DC = D // 128   # 8 d-chunks
JC = D // 128   # 8 j-chunks
SBLK = 512      # s-block size
NSB = SL // SBLK  # 4 s-blocks
NST = SBLK // 128  # 4 s-tiles per block


def r32(ap):
    return ap.bitcast(FP32R)


def build_program(loop_n=1):
    nc = bacc.Bacc(
        "TRN2", target_bir_lowering=False, debug=False,
        enable_asserts=False, num_devices=1,
    )
    q_d = nc.dram_tensor("q", [B, D], FP32, kind="ExternalInput").ap()
    keys_d = nc.dram_tensor("keys", [SL, B, D], FP32, kind="ExternalInput").ap()
    values_d = nc.dram_tensor("values", [SL, B, D], FP32R, kind="ExternalInput").ap()
    wq_d = nc.dram_tensor("wq", [D, D], FP32, kind="ExternalInput").ap()
    wk_d = nc.dram_tensor("wk", [D, D], FP32R, kind="ExternalInput").ap()
    wv_d = nc.dram_tensor("wv", [D, D], FP32, kind="ExternalInput").ap()
    wo_d = nc.dram_tensor("wo", [D, D], FP32, kind="ExternalInput").ap()
    out_d = nc.dram_tensor("out", [B, D], FP32, kind="ExternalOutput").ap()
    ident_d = nc.inline_tensor(np.eye(128, dtype=np.float32), "ident").ap()
    # mask[p, jc, h] = 1/8 if head(jc*128+p) == h else 0
    mask_np = np.zeros((128, JC, H), dtype=np.float32)
    for jc in range(JC):
        for p in range(128):
            mask_np[p, jc, (jc * 128 + p) // NHID] = 0.125
    mask_d = nc.inline_tensor(mask_np, "headmask").ap()

    with tile.TileContext(nc) as tc:
        if loop_n > 1:
            with tc.For_i(0, loop_n, 1):
                _body(tc, out_d, q_d, keys_d, values_d, wq_d, wk_d, wv_d, wo_d,
                      ident_d, mask_d)
        else:
            _body(tc, out_d, q_d, keys_d, values_d, wq_d, wk_d, wv_d, wo_d,
                  ident_d, mask_d)
    nc.compile()
    return nc


def _body(tc, out_d, q_d, keys_d, values_d, wq_d, wk_d, wv_d, wo_d, ident_d, mask_d):
    nc = tc.nc
    from contextlib import ExitStack

    ctx = ExitStack()
    with ctx:
        # ---- persistent pools -------------------------------------------
        const = ctx.enter_context(tc.tile_pool(name="const", bufs=1))
        psum_tr = ctx.enter_context(tc.tile_pool(name="psum_tr", bufs=4, space="PSUM"))
        psum_acc = ctx.enter_context(tc.tile_pool(name="psum_acc", bufs=2, space="PSUM"))
        psum_pv = ctx.enter_context(tc.tile_pool(name="psum_pv", bufs=1, space="PSUM"))

        ident = const.tile([128, 128], FP32, name="ident_sb")
        nc.sync.dma_start(ident[:], ident_d)
        mask_sb = const.tile([128, JC, H], FP32, name="mask_sb")
        nc.sync.dma_start(mask_sb[:], mask_d)

        # rT[p, dc, b, h] = r[b, h, dc*128+p]  (r includes the 1/8 scale)
        rT = const.tile([128, DC, B, H], FP32, name="rT")
        # wvT[p, dc, j] = Wv[j, dc*128+p]
        wvT = const.tile([128, DC, D], FP32, name="wvT")
        # woT[p, jc, i] = Wo[i, jc*128+p]
        woT = const.tile([128, JC, D], FP32, name="woT")
        # zT[p, dc, b, h] = z[b, h, dc*128+p]
        zT = const.tile([128, DC, B, H], FP32, name="zT")
        # attnT[p, jc, b] = attn_flat[b, jc*128+p]
        attnT = const.tile([128, JC, B], FP32, name="attnT")
        out_sb = const.tile([B, D], FP32, name="out_sb")

        # ---- preamble: qp, r --------------------------------------------
        with tc.tile_pool(name="pre", bufs=2) as pre, \
             tc.tile_pool(name="pre1", bufs=1) as pre1:
            q_sb = pre1.tile([B, D], FP32, name="q_sb")
            nc.sync.dma_start(q_sb[:], q_d)
            # qT[p, dc, b] = query[b, dc*128+p]
            qT = pre1.tile([128, DC, B], FP32, name="qT")
            for dc in range(DC):
                ps = psum_tr.tile([128, B], FP32, tag="tr", name=f"ps_qT{dc}")
                nc.tensor.transpose(ps[:], q_sb[:, dc * 128:(dc + 1) * 128],
                                    ident[:B, :B])
                nc.vector.tensor_copy(r32(qT[:, dc, :]), ps[:])

            # wqT[p, dc, j] = Wq[j, dc*128+p]
            wqT = pre1.tile([128, DC, D], FP32, name="wqT")
            for jc in range(JC):
                wq_t = pre.tile([128, D], FP32, tag="w_nat", name=f"wq{jc}")
                nc.sync.dma_start(wq_t[:], wq_d[jc * 128:(jc + 1) * 128, :])
                for dc in range(DC):
                    ps = psum_tr.tile([128, 128], FP32, tag="tr", name=f"ps_wq{jc}_{dc}")
                    nc.tensor.transpose(ps[:], wq_t[:, dc * 128:(dc + 1) * 128],
                                        ident[:])
                    eng = nc.vector.tensor_copy if (jc + dc) % 2 else nc.scalar.copy
                    eng(r32(wqT[:, dc, jc * 128:(jc + 1) * 128]), ps[:])

            # qpT[p, jc, b] = qp[b, jc*128+p]
            qpT = pre1.tile([128, JC, B], FP32, name="qpT")
            for jc in range(JC):
                ps = psum_acc.tile([128, B], FP32, tag="acc", name=f"ps_qp{jc}")
                for dc in range(DC):
                    nc.tensor.matmul(
                        ps[:], r32(wqT[:, dc, jc * 128:(jc + 1) * 128]),
                        r32(qT[:, dc, :]), start=(dc == 0), stop=(dc == DC - 1))
                nc.vector.tensor_copy(qpT[:, jc, :], ps[:])

            # Q[p, jc, b, h] = qp[b, jc*128+p]/8 if head(jc*128+p)==h else 0
            Q = pre1.tile([128, JC, B, H], FP32, name="Q")
            nc.vector.tensor_tensor(
                r32(Q[:]),
                qpT[:, :, :, None].to_broadcast((128, JC, B, H)),
                mask_sb[:, :, None, :].to_broadcast((128, JC, B, H)),
                mybir.AluOpType.mult)

            # wk natural: wk_sb[p, jc, d] = Wk[jc*128+p, d]
            wk_sb = pre1.tile([128, JC, D], FP32R, name="wk_sb")
            for jc in range(JC):
                nc.sync.dma_start(wk_sb[:, jc, :], wk_d[jc * 128:(jc + 1) * 128, :])
            # rT[d, c] = sum_j Wk[j, d] * Q[j, c]
            for dc in range(DC):
                ps = psum_acc.tile([128, B * H], FP32, tag="acc", name=f"ps_r{dc}")
                for jc in range(JC):
                    nc.tensor.matmul(
                        ps[:], wk_sb[:, jc, dc * 128:(dc + 1) * 128],
                        r32(Q[:, jc, :, :]), start=(jc == 0), stop=(jc == JC - 1))
                nc.vector.tensor_copy(r32(rT[:, dc, :, :]), ps[:])

        # ---- Wv / Wo transposes (independent; scheduler fills gaps) -----
        with tc.tile_pool(name="wvo", bufs=2) as wvo:
            for jc in range(JC):
                wv_t = wvo.tile([128, D], FP32, tag="wv_nat", name=f"wv{jc}")
                nc.sync.dma_start(wv_t[:], wv_d[jc * 128:(jc + 1) * 128, :])
                for dc in range(DC):
                    ps = psum_tr.tile([128, 128], FP32, tag="tr", name=f"ps_wv{jc}_{dc}")
                    nc.tensor.transpose(ps[:], wv_t[:, dc * 128:(dc + 1) * 128],
                                        ident[:])
                    eng = nc.vector.tensor_copy if (jc + dc) % 2 else nc.scalar.copy
                    eng(r32(wvT[:, dc, jc * 128:(jc + 1) * 128]), ps[:])
            for ic in range(DC):
                wo_t = wvo.tile([128, D], FP32, tag="wv_nat", name=f"wo{ic}")
                nc.sync.dma_start(wo_t[:], wo_d[ic * 128:(ic + 1) * 128, :])
                for jc in range(JC):
                    ps = psum_tr.tile([128, 128], FP32, tag="tr", name=f"ps_wo{ic}_{jc}")
                    nc.tensor.transpose(ps[:], wo_t[:, jc * 128:(jc + 1) * 128],
                                        ident[:])
                    eng = nc.vector.tensor_copy if (ic + jc) % 2 else nc.scalar.copy
                    eng(r32(woT[:, jc, ic * 128:(ic + 1) * 128]), ps[:])

        # ---- main loop: stream keys/values ------------------------------
        with tc.tile_pool(name="kv", bufs=3) as kv, \
             tc.tile_pool(name="kTp", bufs=2) as kTp, \
             tc.tile_pool(name="dotp", bufs=2) as dotp, \
             tc.tile_pool(name="d17p", bufs=6) as d17p, \
             tc.tile_pool(name="zp", bufs=4) as zp, \
             tc.tile_pool(name="stats", bufs=1) as stats:
            z_tiles = []
            for b in range(B):
                # Row layout: 0..15 = heads, 16..31 = zero padding, 32 = ones
                # (engine ops need base partition in {0,32,64,96}).
                pv0 = psum_pv.tile([33, 512], FP32, tag="pv0", name=f"pv0_{b}")
                pv1 = psum_pv.tile([33, 512], FP32, tag="pv1", name=f"pv1_{b}")
                dot_b = dotp.tile([33, SL], FP32, tag="dot_b", name=f"dot_{b}")
                nc.vector.memset(dot_b[:], 0.0)
                nc.vector.memset(dot_b[32:33, :], 1.0)
                ncopy = 0
                for sblk in range(NSB):
                    s0 = sblk * SBLK
                    kT = kTp.tile([128, DC, SBLK], FP32, tag="kT", name=f"kT_{b}_{sblk}")
                    for st in range(NST):
                        k_t = kv.tile([128, D], FP32, tag="k", name=f"k_{b}_{sblk}_{st}")
                        nc.sync.dma_start(
                            k_t[:], keys_d[s0 + st * 128: s0 + (st + 1) * 128, b, :])
                        for dc in range(DC):
                            ps = psum_tr.tile([128, 128], FP32, tag="tr",
                                              name=f"ps_k{b}_{sblk}_{st}_{dc}")
                            nc.tensor.transpose(
                                ps[:], k_t[:, dc * 128:(dc + 1) * 128], ident[:])
                            eng = (nc.vector.tensor_copy if ncopy % 2
                                   else nc.scalar.copy)
                            ncopy += 1
                            eng(r32(kT[:, dc, st * 128:(st + 1) * 128]), ps[:])
                    # dot[h, s'] for this block
                    ps_dot = psum_acc.tile([16, SBLK], FP32, tag="acc",
                                           name=f"ps_dot{b}_{sblk}")
                    for dc in range(DC):
                        nc.tensor.matmul(
                            ps_dot[:], r32(rT[:, dc, b, :]), r32(kT[:, dc, :]),
                            start=(dc == 0), stop=(dc == DC - 1))
                    nc.scalar.copy(dot_b[0:16, s0:s0 + SBLK], ps_dot[:])
                    # dotT17 + values + P/V accumulation
                    for st in range(NST):
                        cols = slice(s0 + st * 128, s0 + (st + 1) * 128)
                        ps_t = psum_tr.tile([128, 33], FP32, tag="tr",
                                            name=f"ps_dt{b}_{sblk}_{st}")
                        nc.tensor.transpose(ps_t[:], dot_b[:, cols], ident[:33, :33])
                        d17 = d17p.tile([128, 33], FP32, tag="d17",
                                        name=f"d17_{b}_{sblk}_{st}")
                        nc.vector.tensor_copy(r32(d17[:]), ps_t[:])
                        v_t = kv.tile([128, D], FP32R, tag="v", name=f"v_{b}_{sblk}_{st}")
                        nc.sync.dma_start(
                            v_t[:], values_d[s0 + st * 128: s0 + (st + 1) * 128, b, :])
                        first = (sblk == 0 and st == 0)
                        last = (sblk == NSB - 1 and st == NST - 1)
                        nc.tensor.matmul(pv0[:], r32(d17[:]), v_t[:, 0:512],
                                         start=first, stop=last)
                        nc.tensor.matmul(pv1[:], r32(d17[:]), v_t[:, 512:1024],
                                         start=first, stop=last)
                # ---- per-b epilogue: log-softmax correction -------------
                negm = stats.tile([16, 1], FP32, tag="negm", name=f"negm{b}")
                nc.vector.reduce_max(negm[:], dot_b[0:16, :], axis=X)
                nc.vector.tensor_scalar_mul(negm[:], negm[:], -1.0)
                scratch = stats.tile([16, SL], FP32, tag="scratch", name=f"scr{b}")
                S = stats.tile([16, 1], FP32, tag="S", name=f"S{b}")
                nc.scalar.activation(scratch[:], dot_b[0:16, :], Exp,
                                     bias=negm[:], scale=1.0, accum_out=S[:])
                lnS = stats.tile([16, 1], FP32, tag="lnS", name=f"lnS{b}")
                nc.scalar.activation(lnS[:], S[:], Ln)
                neg_lse = stats.tile([16, 1], FP32, tag="neg_lse", name=f"nlse{b}")
                nc.vector.tensor_tensor(neg_lse[:], negm[:], lnS[:],
                                        mybir.AluOpType.subtract)
                # V = column sums of values (PSUM row 32) -> broadcast to 16
                # partitions, then z = P + (-lse) * V via DVE.
                V_sb = stats.tile([33, D], FP32, tag="V", name=f"V{b}")
                nc.scalar.copy(V_sb[32:33, 0:512], pv0[32:33, :])
                nc.scalar.copy(V_sb[32:33, 512:1024], pv1[32:33, :])
                V0 = stats.tile([1, D], FP32, tag="V0", name=f"V0_{b}")
                nc.sync.dma_start(V0[:], V_sb[32:33, :])
                V16 = stats.tile([16, D], FP32, tag="V16", name=f"V16_{b}")
                nc.gpsimd.partition_broadcast(V16[:], V0[:])
                corr = stats.tile([16, D], FP32, tag="corr", name=f"corr{b}")
                nc.vector.tensor_scalar_mul(corr[:], V16[:], neg_lse[:])
                z_b = zp.tile([16, D], FP32, tag="z", name=f"z{b}")
                nc.vector.tensor_tensor(z_b[:, 0:512], pv0[0:16, :],
                                        corr[:, 0:512], mybir.AluOpType.add)
                nc.vector.tensor_tensor(z_b[:, 512:1024], pv1[0:16, :],
                                        corr[:, 512:1024], mybir.AluOpType.add)
                z_tiles.append(z_b)

            # ---- tail: attn and output projection -----------------------
            for b in range(B):
                for dc in range(DC):
                    ps = psum_tr.tile([128, 16], FP32, tag="tr",
                                      name=f"ps_z{b}_{dc}")
                    nc.tensor.transpose(
                        ps[:], z_tiles[b][:, dc * 128:(dc + 1) * 128],
                        ident[:16, :16])
                    nc.vector.tensor_copy(r32(zT[:, dc, b, :]), ps[:])
            for h in range(H):
                ps_a = psum_acc.tile([64, B], FP32, tag="acc", name=f"ps_a{h}")
                for dc in range(DC):
                    nc.tensor.matmul(
                        ps_a[:], r32(wvT[:, dc, h * 64:(h + 1) * 64]),
                        r32(zT[:, dc, :, h]), start=(dc == 0), stop=(dc == DC - 1))
                jc, half = h // 2, h % 2
                rows = slice(64 * half, 64 * half + 64)
                nc.vector.tensor_copy(r32(attnT[rows, jc, :]), ps_a[:])
            ps_o0 = psum_acc.tile([B, 512], FP32, tag="acc", name="ps_o0")
            ps_o1 = psum_acc.tile([B, 512], FP32, tag="acc", name="ps_o1")
            for jc in range(JC):
                nc.tensor.matmul(ps_o0[:], r32(attnT[:, jc, :]),
                                 r32(woT[:, jc, 0:512]),
                                 start=(jc == 0), stop=(jc == JC - 1))
                nc.tensor.matmul(ps_o1[:], r32(attnT[:, jc, :]),
                                 r32(woT[:, jc, 512:1024]),
                                 start=(jc == 0), stop=(jc == JC - 1))
            nc.vector.tensor_copy(out_sb[:, 0:512], ps_o0[:])
            nc.vector.tensor_copy(out_sb[:, 512:1024], ps_o1[:])
            nc.sync.dma_start(out_d, out_sb[:])


_NC_CACHE = {}


def get_program():
    if "nc" not in _NC_CACHE:
        _NC_CACHE["nc"] = build_program()
    return _NC_CACHE["nc"]


def make_in_maps(query, keys, values, Wq, Wk, Wv, Wo):
    query = np.asarray(query, dtype=np.float32)
    keys = np.asarray(keys, dtype=np.float32)
    values = np.asarray(values, dtype=np.float32)
    Wq = np.ascontiguousarray(np.asarray(Wq, dtype=np.float32))
    Wk = np.ascontiguousarray(np.asarray(Wk, dtype=np.float32))
    Wv = np.ascontiguousarray(np.asarray(Wv, dtype=np.float32))
    Wo = np.ascontiguousarray(np.asarray(Wo, dtype=np.float32))
    in_maps = []
    for i in range(NCORES):
        sl = slice(B * i, B * (i + 1))
        in_maps.append({
            "q": np.ascontiguousarray(query[sl]),
            "keys": np.ascontiguousarray(keys[:, sl, :]),
            "values": np.ascontiguousarray(values[:, sl, :]),
            "wq": Wq, "wk": Wk, "wv": Wv, "wo": Wo,
        })
    return in_maps


def kernel(query, keys, values, Wq, Wk, Wv, Wo):
    nc = get_program()
    in_maps = make_in_maps(query, keys, values, Wq, Wk, Wv, Wo)
    res = bass_utils.run_bass_kernel_spmd(nc, in_maps, core_ids=list(range(NCORES)))
    return np.concatenate(
        [res.results[i]["out"] for i in range(NCORES)], axis=0)
